# revision 21
# baseline (speedup 1.0000x reference)
"""Trainium2 Bass kernel for the MAVE global-epistasis measurement layer.

    y[b] = a_0 + sum_k bk[k] * tanh( (ck @ z[b])[k] + dk[k] )
    z: [2097152, 16] f32, ck: [64, 16], bk, dk: [64], a_0: [1]

Data-parallel over 8 NeuronCores (262144 batch rows per core).

Per-core dataflow (Tile kernel), batch enumerated per super s (4096 rows):
  - z loaded in [128, 512] tiles, partition p = 32 consecutive rows
    (2 KB contiguous per partition -> 128 fat DMA descriptors per tile).
  - DVE 32x32 block transpose -> zt[32a + 16v + z, 32Q + j] =
    z[4096 s + 1024 a + 32 j + 2 Q + v, z]; bitcast to f32r (same bits).
  - TensorE: 4 concurrent row-tiled matmuls (tile_position=(32a,0), K=32)
    against a block-diagonal ck stationary -> pre-h [128=(64v+k), 512].
  - ScalarE: tanh(x + dk) over [128, 1536] PSUM tiles (3 banks) -> bf16.
  - TensorE: bk stationary [128, 32] reduces k; 16 groups (slots)
    accumulate into one y PSUM [32=(2*slot+v), 512] bank.
  - GpSimd adds a_0 (PSUM->SBUF); DVE 32x32-transposes y so each
    partition j holds batch rows {1024 slot + 32 j + (0..31)}; HWDGE
    writes 128-byte contiguous DRAM runs.
"""
import numpy as np

import concourse.bass as bass
import concourse.tile as tile
from concourse import mybir
from concourse.bass_utils import run_bass_kernel_spmd

from contextlib import ExitStack

F32 = mybir.dt.float32
F32R = mybir.dt.float32r
BF16 = mybir.dt.bfloat16
U32 = mybir.dt.uint32

B_FULL = 2097152
N_CORES = 8
NC_ROWS = B_FULL // N_CORES          # 262144
SUPER = 4096                         # rows per transpose tile
N_SUPER = NC_ROWS // SUPER           # 64
N_GROUPS = N_SUPER * 4               # 256  (1024 rows each)
SPAN = 16384                         # rows per y flush (16 groups)
N_SPAN = NC_ROWS // SPAN             # 16
HTILE = 3                            # groups per ACT tanh op (3 PSUM banks)


def _multiwait_split(nc):
    ctr = 0
    for f in nc.m.functions:
        for blk in f.blocks:
            insts = blk.instructions
            i = 0
            while i < len(insts):
                inst = insts[i]
                si = getattr(inst, "sync_info", None)
                if si is not None and si.on_wait and len(si.on_wait) > 1:
                    extra = list(si.on_wait[:-1])
                    del si.on_wait[:-1]
                    for w in extra:
                        ctr += 1
                        nop = mybir.InstNoOp(name=f"I-mws-{ctr}", ins=[], outs=[])
                        nop.engine = inst.engine
                        nop.sync_info = mybir.SyncInfo(on_wait=[w], on_update=[])
                        insts.insert(i, nop)
                        i += 1
                i += 1
    return nc


def build_nc():
    nc = bass.Bass()
    z_ext = nc.declare_dram_parameter("z", [NC_ROWS, 16], F32, isOutput=False)
    a0_ext = nc.declare_dram_parameter("a_0", [1], F32, isOutput=False)
    bk_ext = nc.declare_dram_parameter("bk", [64], F32, isOutput=False)
    ck_ext = nc.declare_dram_parameter("ck", [64, 16], F32, isOutput=False)
    dk_ext = nc.declare_dram_parameter("dk", [64], F32, isOutput=False)
    y_ext = nc.declare_dram_parameter("y", [NC_ROWS, 1], F32, isOutput=True)

    ctx = ExitStack()
    with ctx:
        tc = ctx.enter_context(tile.TileContext(nc))
        consts = ctx.enter_context(tc.tile_pool(name="consts", bufs=1))
        zn_pool = ctx.enter_context(tc.tile_pool(name="zn", bufs=3))
        zt_pool = ctx.enter_context(tc.tile_pool(name="zt", bufs=2))
        ztr_pool = ctx.enter_context(tc.tile_pool(name="ztr", bufs=3))
        hsb_pool = ctx.enter_context(tc.tile_pool(name="hsb", bufs=3))
        ysb_pool = ctx.enter_context(tc.tile_pool(name="ysb", bufs=2))
        yt_pool = ctx.enter_context(tc.tile_pool(name="yt", bufs=2))
        hps_pool = ctx.enter_context(tc.tile_pool(name="hps", bufs=2, space="PSUM"))
        yps_pool = ctx.enter_context(tc.tile_pool(name="yps", bufs=2, space="PSUM"))

        # ---- constants -------------------------------------------------
        ckT = ck_ext[:].rearrange("k z -> z k")          # [16, 64] strided view
        # Block-diagonal ck stationary: 4 identical [32, 128] row strips;
        # strip rows 16v+z, cols 64v+k hold ckT.
        ckbd = consts.tile([128, 128], BF16, tag="ckbd")
        nc.vector.memset(ckbd.bitcast(mybir.dt.uint16), 0)
        for a in range(4):
            for v in range(2):
                nc.gpsimd.dma_start(
                    out=ckbd[32 * a + 16 * v: 32 * a + 16 * v + 16,
                             64 * v: 64 * v + 64],
                    in_=ckT,
                )

        bw = []
        for slot in range(16):
            t = consts.tile([128, 32], BF16, tag=f"bw{slot}")
            nc.vector.memset(t.bitcast(mybir.dt.uint16), 0)
            for v in range(2):
                m = 2 * slot + v
                nc.gpsimd.dma_start(
                    out=t[64 * v: 64 * v + 64, m: m + 1],
                    in_=bk_ext[:],
                )
            bw.append(t)

        dk_col = consts.tile([128, 1], F32, tag="dkcol")
        for v in range(2):
            nc.gpsimd.dma_start(out=dk_col[64 * v: 64 * v + 64, :], in_=dk_ext[:])
        a0_col = consts.tile([32, 1], F32, tag="a0col")
        nc.gpsimd.dma_start(out=a0_col, in_=a0_ext[:].to_broadcast((32, 1)))

        # DRAM views ------------------------------------------------------
        # z input: partition p of super s holds rows [4096 s + 32 p, +32),
        # i.e. 2 KB contiguous per partition.
        zd = z_ext[:].rearrange("(s p r) z -> s p (r z)", s=N_SUPER, p=128, r=32)
        # y output: row = 16384 t + 1024 slot + 32 j + 2 Q + v; SBUF source
        # yt[j, 32 Q + 2 slot + v] -> contiguous (Q v) runs of 128 B.
        # row = 16384 t + 1024 slot + 32 j + w, where w = 2Q + v
        yd = y_ext[:].rearrange(
            "(t slot j w) one -> t j slot (w one)",
            t=N_SPAN, slot=16, j=32, w=32,
        )

        # ---- PE warmup: ~4 us of dummy matmuls so the HAM clock-gate
        # reaches 8/8 before the first real h-matmul (PE duty stays high
        # enough afterwards that it never re-throttles).
        warm_ps = yps_pool.tile([32, 512], F32, tag="y_ps")
        for _ in range(30):
            nc.tensor.matmul(
                warm_ps[:, :128], bw[0], ckbd, start=True, stop=True,
            )

        # ---- main loop ---------------------------------------------------
        h_ps = h_sb = y_ps = None
        pend = []

        def flush_act():
            nonlocal pend, y_ps
            if not pend:
                return
            ncols = len(pend) * 512
            nc.scalar.activation(
                h_sb[:, :ncols], h_ps[:, :ncols],
                mybir.ActivationFunctionType.Tanh,
                bias=dk_col, scale=1.0,
            )
            for gg, col in pend:
                slot = gg % 16
                if slot == 0:
                    y_ps = yps_pool.tile([32, 512], F32)
                nc.tensor.matmul(
                    y_ps, bw[slot], h_sb[:, col:col + 512],
                    start=(slot == 0), stop=(slot == 15),
                )
                if slot == 15:
                    t = gg // 16
                    # y_tr[j, 32 Q + 2 slot + v] = y_ps[2 slot + v, 32 Q + j]
                    y_tr = yt_pool.tile([32, 512], F32)
                    nc.vector.transpose(y_tr, y_ps)
                    # y_fin[j, 32 slot + 2 Q + v] = y_tr[j, 32 Q + 2 slot + v] + a0
                    y_fin = ysb_pool.tile([32, 512], F32)
                    nc.vector.tensor_scalar_add(
                        y_fin.rearrange("j (slot Q v) -> j slot Q v",
                                        slot=16, Q=16, v=2),
                        y_tr.rearrange("j (Q slot v) -> j slot Q v",
                                       Q=16, slot=16, v=2),
                        a0_col,
                    )
                    nc.sync.dma_start(
                        out=yd[t],
                        in_=y_fin.rearrange("j (slot w) -> j slot w",
                                            slot=16, w=32),
                    )
            pend = []

        for g in range(N_GROUPS):
            s, a = divmod(g, 4)
            if a == 0:
                zb = zn_pool.tile([128, 512], BF16)
                nc.gpsimd.dma_start(out=zb, in_=zd[s])
                zt = ztr_pool.tile([128, 512], BF16)
                nc.vector.transpose(zt, zb)
            if g % HTILE == 0:
                h_ps = hps_pool.tile([128, HTILE * 512], F32)
                h_sb = hsb_pool.tile([128, HTILE * 512], BF16)
            col = (g % HTILE) * 512
            nc.tensor.matmul(
                h_ps[:, col:col + 512],
                ckbd[32 * a:32 * a + 32, :],
                zt[32 * a:32 * a + 32, :],
                start=True, stop=True,
                tile_position=(32 * a, 0),
            )
            pend.append((g, col))
            if g % HTILE == HTILE - 1 or g == N_GROUPS - 1:
                flush_act()

    _multiwait_split(nc)
    return nc


_NC_CACHE = None


def _get_nc():
    global _NC_CACHE
    if _NC_CACHE is None:
        _NC_CACHE = build_nc()
    return _NC_CACHE


def _run(inputs, **run_kwargs):
    nc = _get_nc()
    z = np.ascontiguousarray(np.asarray(inputs["z"], dtype=np.float32))
    a0 = np.asarray(inputs["a_0"], dtype=np.float32).reshape(1)
    bk = np.asarray(inputs["bk"], dtype=np.float32).reshape(64)
    ck = np.ascontiguousarray(np.asarray(inputs["ck"], dtype=np.float32))
    dk = np.asarray(inputs["dk"], dtype=np.float32).reshape(64)
    in_maps = []
    for c in range(N_CORES):
        in_maps.append({
            "z": z[c * NC_ROWS:(c + 1) * NC_ROWS],
            "a_0": a0, "bk": bk, "ck": ck, "dk": dk,
        })
    res = run_bass_kernel_spmd(nc, in_maps, core_ids=list(range(N_CORES)),
                               **run_kwargs)
    y = np.concatenate([res.results[c]["y"] for c in range(N_CORES)], axis=0)
    return y, res


def kernel(**inputs) -> np.ndarray:
    y, _ = _run(inputs)
    return y


# revision 27
# speedup vs baseline: 1.1289x; 1.1289x over previous
"""Trainium2 Bass kernel for the MAVE global-epistasis measurement layer.

    y[b] = a_0 + sum_k bk[k] * tanh( (ck @ z[b])[k] + dk[k] )
    z: [2097152, 16] f32, ck: [64, 16], bk, dk: [64], a_0: [1]

Data-parallel over 8 NeuronCores (262144 batch rows per core).

Per-core dataflow (Tile kernel), batch enumerated per super s (4096 rows):
  - z loaded in [128, 512] tiles, partition p = 32 consecutive rows
    (2 KB contiguous per partition -> 128 fat DMA descriptors per tile).
  - DVE 32x32 block transpose -> zt[32a + 16v + z, 32Q + j] =
    z[4096 s + 1024 a + 32 j + 2 Q + v, z]; bitcast to f32r (same bits).
  - TensorE: 4 concurrent row-tiled matmuls (tile_position=(32a,0), K=32)
    against a block-diagonal ck stationary -> pre-h [128=(64v+k), 512].
  - ScalarE: tanh(x + dk) over [128, 1536] PSUM tiles (3 banks) -> bf16.
  - TensorE: bk stationary [128, 32] reduces k; 16 groups (slots)
    accumulate into one y PSUM [32=(2*slot+v), 512] bank.
  - GpSimd adds a_0 (PSUM->SBUF); DVE 32x32-transposes y so each
    partition j holds batch rows {1024 slot + 32 j + (0..31)}; HWDGE
    writes 128-byte contiguous DRAM runs.
"""
import numpy as np

import concourse.bass as bass
import concourse.tile as tile
from concourse import mybir
from concourse.bass_utils import run_bass_kernel_spmd

from contextlib import ExitStack

F32 = mybir.dt.float32
F32R = mybir.dt.float32r
BF16 = mybir.dt.bfloat16
U32 = mybir.dt.uint32

B_FULL = 2097152
N_CORES = 8
NC_ROWS = B_FULL // N_CORES          # 262144
SUPER = 4096                         # rows per transpose tile
N_SUPER = NC_ROWS // SUPER           # 64
N_GROUPS = N_SUPER * 4               # 256  (1024 rows each)
SPAN = 16384                         # rows per y flush (16 groups)
N_SPAN = NC_ROWS // SPAN             # 16
HTILE = 3                            # groups per ACT tanh op (3 PSUM banks)


def _multiwait_split(nc):
    ctr = 0
    for f in nc.m.functions:
        for blk in f.blocks:
            insts = blk.instructions
            i = 0
            while i < len(insts):
                inst = insts[i]
                si = getattr(inst, "sync_info", None)
                if si is not None and si.on_wait and len(si.on_wait) > 1:
                    extra = list(si.on_wait[:-1])
                    del si.on_wait[:-1]
                    for w in extra:
                        ctr += 1
                        nop = mybir.InstNoOp(name=f"I-mws-{ctr}", ins=[], outs=[])
                        nop.engine = inst.engine
                        nop.sync_info = mybir.SyncInfo(on_wait=[w], on_update=[])
                        insts.insert(i, nop)
                        i += 1
                i += 1
    return nc


def build_nc():
    nc = bass.Bass()
    z_ext = nc.declare_dram_parameter("z", [NC_ROWS, 16], F32, isOutput=False)
    a0_ext = nc.declare_dram_parameter("a_0", [1], F32, isOutput=False)
    bk_ext = nc.declare_dram_parameter("bk", [64], F32, isOutput=False)
    ck_ext = nc.declare_dram_parameter("ck", [64, 16], F32, isOutput=False)
    dk_ext = nc.declare_dram_parameter("dk", [64], F32, isOutput=False)
    y_ext = nc.declare_dram_parameter("y", [NC_ROWS, 1], F32, isOutput=True)

    ctx = ExitStack()
    with ctx:
        tc = ctx.enter_context(tile.TileContext(nc))
        consts = ctx.enter_context(tc.tile_pool(name="consts", bufs=1))
        zn_pool = ctx.enter_context(tc.tile_pool(name="zn", bufs=3))
        zt_pool = ctx.enter_context(tc.tile_pool(name="zt", bufs=2))
        ztr_pool = ctx.enter_context(tc.tile_pool(name="ztr", bufs=3))
        hsb_pool = ctx.enter_context(tc.tile_pool(name="hsb", bufs=3))
        ysb_pool = ctx.enter_context(tc.tile_pool(name="ysb", bufs=2))
        yt_pool = ctx.enter_context(tc.tile_pool(name="yt", bufs=2))
        hps_pool = ctx.enter_context(tc.tile_pool(name="hps", bufs=2, space="PSUM"))
        yps_pool = ctx.enter_context(tc.tile_pool(name="yps", bufs=2, space="PSUM"))

        # DRAM views ------------------------------------------------------
        # z input: partition p of super s holds rows [4096 s + 32 p, +32),
        # i.e. 2 KB contiguous per partition.
        zd = z_ext[:].rearrange("(s p r) z -> s p (r z)", s=N_SUPER, p=128, r=32)
        # row = 16384 t + 1024 slot + 32 j + w, where w = 2Q + v
        yd = y_ext[:].rearrange(
            "(t slot j w) one -> t j slot (w one)",
            t=N_SPAN, slot=16, j=32, w=32,
        )

        # ---- z prefetch (issued ahead of consts on the gpsimd DGE queue,
        # so the first transposes/matmuls are not gated on ~10 const DMAs).
        zb_tiles = {}

        def load_super(s):
            zb = zn_pool.tile([128, 512], BF16, tag="zb")
            nc.gpsimd.dma_start(out=zb, in_=zd[s])
            zb_tiles[s] = zb

        for s in range(3):
            load_super(s)

        # ---- constants -------------------------------------------------
        # Block-diagonal ck stationary: 4 identical [32, 128] row strips;
        # strip rows 16v+z, cols 64v+k hold ckT. One broadcast DMA per v.
        ckbd = consts.tile([128, 128], BF16, tag="ckbd")
        nc.vector.memset(ckbd.bitcast(mybir.dt.uint16), 0)
        ckT = ck_ext[:].rearrange("k z -> z k")          # [16, 64] strided view
        for a in range(4):
            for v in range(2):
                nc.gpsimd.dma_start(
                    out=ckbd[32 * a + 16 * v: 32 * a + 16 * v + 16,
                             64 * v: 64 * v + 64],
                    in_=ckT,
                )

        # bk stationary as one wide tile; slot s uses cols [30-2s, 62-2s),
        # placing bk (col 30 for v=0 rows, col 31 for v=1 rows) at
        # slice-local cols 2s / 2s+1 -> y partitions 2s / 2s+1.
        bwide = consts.tile([128, 64], BF16, tag="bwide")
        nc.vector.memset(bwide.bitcast(mybir.dt.uint16), 0)
        nc.gpsimd.dma_start(out=bwide[0:64, 30:31], in_=bk_ext[:])
        nc.gpsimd.dma_start(out=bwide[64:128, 31:32], in_=bk_ext[:])
        bw = [bwide[:, 30 - 2 * s: 62 - 2 * s] for s in range(16)]

        dk_col = consts.tile([128, 1], F32, tag="dkcol")
        for v in range(2):
            nc.sync.dma_start(out=dk_col[64 * v: 64 * v + 64, :], in_=dk_ext[:])
        a0_col = consts.tile([32, 1], F32, tag="a0col")
        nc.sync.dma_start(out=a0_col, in_=a0_ext[:].to_broadcast((32, 1)))

        # Force the tanh ACT table load during startup (it is otherwise
        # lazily loaded at the first real activation, ~1.3 us mid-pipeline).
        tanh_warm = consts.tile([128, 1], BF16, tag="tanhwarm")
        nc.scalar.activation(tanh_warm, dk_col,
                             mybir.ActivationFunctionType.Tanh,
                             bias=dk_col, scale=1.0)

        # ---- PE warmup: ~4 us of dummy matmuls so the HAM clock-gate
        # reaches 8/8 before the first real h-matmul (PE duty stays high
        # enough afterwards that it never re-throttles).
        warm_ps = yps_pool.tile([32, 512], F32, tag="y_ps")
        for _ in range(30):
            nc.tensor.matmul(
                warm_ps[:, :128], bw[0], ckbd, start=True, stop=True,
            )

        # ---- main loop ---------------------------------------------------
        h_ps = h_sb = y_ps = None
        pend = []

        def flush_act():
            nonlocal pend, y_ps
            if not pend:
                return
            ncols = len(pend) * 512
            nc.scalar.activation(
                h_sb[:, :ncols], h_ps[:, :ncols],
                mybir.ActivationFunctionType.Tanh,
                bias=dk_col, scale=1.0,
            )
            for gg, col in pend:
                slot = gg % 16
                if slot == 0:
                    y_ps = yps_pool.tile([32, 512], F32)
                nc.tensor.matmul(
                    y_ps, bw[slot], h_sb[:, col:col + 512],
                    start=(slot == 0), stop=(slot == 15),
                )
                if slot == 15:
                    t = gg // 16
                    # y_tr[j, 32 Q + 2 slot + v] = y_ps[2 slot + v, 32 Q + j]
                    y_tr = yt_pool.tile([32, 512], F32)
                    nc.vector.transpose(y_tr, y_ps)
                    # y_fin[j, 32 slot + 2 Q + v] = y_tr[j, 32 Q + 2 slot + v] + a0
                    y_fin = ysb_pool.tile([32, 512], F32)
                    nc.vector.tensor_scalar_add(
                        y_fin.rearrange("j (slot Q v) -> j slot Q v",
                                        slot=16, Q=16, v=2),
                        y_tr.rearrange("j (Q slot v) -> j slot Q v",
                                       Q=16, slot=16, v=2),
                        a0_col,
                    )
                    nc.sync.dma_start(
                        out=yd[t],
                        in_=y_fin.rearrange("j (slot w) -> j slot w",
                                            slot=16, w=32),
                    )
            pend = []

        for g in range(N_GROUPS):
            s, a = divmod(g, 4)
            if a == 0:
                if s not in zb_tiles:
                    load_super(s)
                zb = zb_tiles.pop(s)
                if s + 3 < N_SUPER:
                    load_super(s + 3)
                zt = ztr_pool.tile([128, 512], BF16)
                nc.vector.transpose(zt, zb)
            if g % HTILE == 0:
                h_ps = hps_pool.tile([128, HTILE * 512], F32)
                h_sb = hsb_pool.tile([128, HTILE * 512], BF16)
            col = (g % HTILE) * 512
            nc.tensor.matmul(
                h_ps[:, col:col + 512],
                ckbd[32 * a:32 * a + 32, :],
                zt[32 * a:32 * a + 32, :],
                start=True, stop=True,
                tile_position=(32 * a, 0),
            )
            pend.append((g, col))
            if g % HTILE == HTILE - 1 or g == N_GROUPS - 1:
                flush_act()

    _multiwait_split(nc)
    return nc


_NC_CACHE = None


def _get_nc():
    global _NC_CACHE
    if _NC_CACHE is None:
        _NC_CACHE = build_nc()
    return _NC_CACHE


def _run(inputs, **run_kwargs):
    nc = _get_nc()
    z = np.ascontiguousarray(np.asarray(inputs["z"], dtype=np.float32))
    a0 = np.asarray(inputs["a_0"], dtype=np.float32).reshape(1)
    bk = np.asarray(inputs["bk"], dtype=np.float32).reshape(64)
    ck = np.ascontiguousarray(np.asarray(inputs["ck"], dtype=np.float32))
    dk = np.asarray(inputs["dk"], dtype=np.float32).reshape(64)
    in_maps = []
    for c in range(N_CORES):
        in_maps.append({
            "z": z[c * NC_ROWS:(c + 1) * NC_ROWS],
            "a_0": a0, "bk": bk, "ck": ck, "dk": dk,
        })
    res = run_bass_kernel_spmd(nc, in_maps, core_ids=list(range(N_CORES)),
                               **run_kwargs)
    y = np.concatenate([res.results[c]["y"] for c in range(N_CORES)], axis=0)
    return y, res


def kernel(**inputs) -> np.ndarray:
    y, _ = _run(inputs)
    return y
